# revision 21
# baseline (speedup 1.0000x reference)
"""Trainium2 Bass kernel for nn_DCModule_25451976196444.

Sliding-window (3x3, stride 2) min/max-|anchor-comp| selection pooling:
for each window, pick the comp value where |anchor-comp| is minimal and
where it is maximal; output = sum of the two, broadcast over the window
footprint (last covering window wins).

v3: both comparisons (positive & negative) are batched into every
element-wise op (halves instruction count vs per-comp loops), the L1
horizontal mask is shared between the min and max chains (strict-tie
deviation only, measure-zero on continuous data), |x| and select-seeds
run on the Scalar engine, subtracts on GpSimd, and outputs are stored as
fp16 (halves write traffic; ~2e-4 rel-L2 from quantization, gate 2e-2).
Work is organized in 4 load-units (2 row-blocks x 2 width-halves, ~2MB
DMAs); each unit's load+subtract+abs is emitted one unit AHEAD so the
scalar engine's program order never stalls the vector engine between
iterations.  Stores go through SWDGE (gpsimd.dma_start) which spreads
descriptors across all 16 SDMA engines.

Per core (rows sharded across 8 cores):
  - pair tiles [128, 2, cols]: partition p holds image rows (2p, 2p+1)
  - horizontal pass merges the 3 column candidates per window in f32
    (compares must be f32-exact: fp16 compares flip ~0.5% of windows,
    which fails the error gate)
  - the third vertical candidate (row 2i+2) is the even-plane H-result
    shifted by one partition via TensorE subdiagonal-identity matmul
    (PSUM comp-stride padded to 512 so both matmuls are bank-aligned)
  - vertical pass merges the 3 row candidates; min+max selections are
    summed and column+row duplicated into fp16 tiles for the store
Each core computes 254 of its 256 window-rows; the host computes the last
2 window-rows per core plus the uncovered boundary rows/cols in numpy with
identical f32 semantics.
"""

import numpy as np
from contextlib import ExitStack

import concourse.bass as bass
import concourse.mybir as mybir
import concourse.tile as tile
from concourse import bacc
from concourse import bass_utils
from concourse._compat import with_exitstack

F32 = mybir.dt.float32
F16 = mybir.dt.float16
U8 = mybir.dt.uint8
U32 = mybir.dt.uint32
ALU = mybir.AluOpType
ACTF = mybir.ActivationFunctionType

H = 4096
W = 4096
WS = 3
ST = 2
NCORES = 8
BP = 128                    # partitions per row-block (pair tiles)
NJT = 512                   # window-cols per column tile


def _geom():
    """(Re)compute derived geometry from H/W/BP/NJT (tests patch these)."""
    global OUTR, SLAB, VR, NJ_TOT, VBLK, JTILES, JOFFS, OUT_ELEMS, BLOCKS
    global UNITS, UW
    OUTR = H // NCORES
    SLAB = OUTR
    VR = OUTR // 2
    NJ_TOT = (W - WS) // ST + 1
    VBLK = BP - 1
    assert VR == 2 * VBLK + 2, (VR, VBLK)
    JTILES = []
    j0 = 0
    while j0 < NJ_TOT:
        JTILES.append((j0, min(NJT, NJ_TOT - j0)))
        j0 += NJT
    JOFFS = []
    off = 0
    for (_j, _nj) in JTILES:
        JOFFS.append(off)
        off += OUTR * 2 * _nj
    OUT_ELEMS = off
    BLOCKS = [(0, VBLK), (VBLK, VBLK)]   # device window-rows 0..2*VBLK-1
    # load-units: one (block, jtile) per unit so the working set fits SBUF
    UNITS = []
    for (i0, nb) in BLOCKS:
        for ct in range(len(JTILES)):
            UNITS.append((i0, nb, [ct]))
    UW = max(2 * nj + 1 for (_j, nj) in JTILES)


_geom()


def _emit(ctx: ExitStack, tc, a, p, n, smat, outp, outn):
    nc = tc.nc

    ina_pool = ctx.enter_context(tc.tile_pool(name="ina", bufs=1))
    in_pool = ctx.enter_context(tc.tile_pool(name="in", bufs=2))
    x_pool = ctx.enter_context(tc.tile_pool(name="x", bufs=1))
    d_pool = ctx.enter_context(tc.tile_pool(name="d", bufs=2))
    h_pool = ctx.enter_context(tc.tile_pool(name="h", bufs=1))
    m_pool = ctx.enter_context(tc.tile_pool(name="m", bufs=1))
    v_pool = ctx.enter_context(tc.tile_pool(name="v", bufs=1))
    o_pool = ctx.enter_context(tc.tile_pool(name="o", bufs=2))
    c_pool = ctx.enter_context(tc.tile_pool(name="c", bufs=1))
    ps_pool = ctx.enter_context(tc.tile_pool(name="ps", bufs=1, space="PSUM"))

    sm = c_pool.tile([BP, BP], F32, tag="sm")
    nc.sync.dma_start(sm[:], smat[:])

    def unit_cols(u):
        i0, nb, jts = UNITS[u]
        c0 = 2 * JTILES[jts[0]][0]
        cend = 2 * JTILES[jts[-1]][0] + 2 * JTILES[jts[-1]][1] + 1
        return c0, cend - c0

    def stage_load(u):
        """DMA loads for unit u (emitted one unit ahead)."""
        i0, nb, jts = UNITS[u]
        c0, ucw = unit_cols(u)
        rr = slice(2 * i0, 2 * i0 + 2 * BP)
        cs = slice(c0, c0 + ucw)
        A_ = ina_pool.tile([BP, 2, UW], F32, tag="A")
        C_ = in_pool.tile([BP, 2, 2, UW], F32, tag="C")
        nc.sync.dma_start(
            A_[:, :, 0:ucw], a[rr, cs].rearrange("(q t) w -> q t w", t=2))
        nc.sync.dma_start(
            C_[:, 0, :, 0:ucw], p[rr, cs].rearrange("(q t) w -> q t w", t=2))
        nc.sync.dma_start(
            C_[:, 1, :, 0:ucw], n[rr, cs].rearrange("(q t) w -> q t w", t=2))
        return A_, C_

    def stage_sub(u, A_, C_):
        """x = a - c (GpSimd) and d = |x| (Scalar) for unit u; emitted after
        the current unit's delta-masks so GpSimd produces those first."""
        _i0, _nb, _jts = UNITS[u]
        _c0, ucw = unit_cols(u)
        xx = x_pool.tile([BP, 2, 2, UW], F32, tag="xx")
        nc.gpsimd.tensor_tensor(
            xx[:, 0, :, 0:ucw], A_[:, :, 0:ucw], C_[:, 0, :, 0:ucw],
            op=ALU.subtract)
        nc.gpsimd.tensor_tensor(
            xx[:, 1, :, 0:ucw], A_[:, :, 0:ucw], C_[:, 1, :, 0:ucw],
            op=ALU.subtract)
        dd = d_pool.tile([BP, 2, 2, UW], F32, tag="dd")
        nc.scalar.activation(dd[:, :, :, 0:ucw], xx[:, :, :, 0:ucw], ACTF.Abs)
        return dd

    _l0 = stage_load(0)
    staged = (_l0[1], stage_sub(0, *_l0))
    next_load = stage_load(1) if len(UNITS) > 1 else None
    pending = []   # deferred output stage: (ss, nb, i0, ct) emitted next unit

    def flush_out():
        """Column+row duplicate into fp16 and store via SWDGE."""
        while pending:
            ss, nb, i0, ct = pending.pop(0)
            (j0, nj) = JTILES[ct]
            w = 2 * nj
            vv = o_pool.tile([nb, 2, 2, w], F16, tag="vv")
            nc.scalar.copy(vv[:, :, 0, 0:w - 1:2], ss[:])
            nc.scalar.copy(vv[:, :, 0, 1:w:2], ss[:])
            nc.scalar.copy(vv[:, :, 1, :], vv[:, :, 0, :])
            base = JOFFS[ct] + 2 * i0 * w
            for ci, OUT in ((0, outp), (1, outn)):
                dst = OUT[base:base + 2 * nb * w].rearrange(
                    "(r w) -> r w", w=w)
                nc.gpsimd.dma_start(dst, vv[:, ci])

    for u, (i0, nb, jts) in enumerate(UNITS):
        C_, dd = staged
        flush_out()
        uc0, _ucw = unit_cols(u)

        for ct in jts:
            (j0, nj) = JTILES[ct]
            lc0 = 2 * j0 - uc0          # col offset within the unit tiles
            w = 2 * nj
            s0 = slice(lc0 + 0, lc0 + w - 1, 2)
            s1 = slice(lc0 + 1, lc0 + w, 2)
            s2 = slice(lc0 + 2, lc0 + w + 1, 2)

            # ---- horizontal pass: merge 3 col candidates, both chains ----
            # layout [BP, comp, plane, nj]; candidate order v=0,1,2 with the
            # L1 mask shared between chains (ties deviate, harmless).
            mk1 = m_pool.tile([BP, 2, 2, nj], U8, tag="mk1")
            dlM = m_pool.tile([BP, 2, 2, nj], F32, tag="dlM")
            dlm = m_pool.tile([BP, 2, 2, nj], F32, tag="dlm")
            hdM = h_pool.tile([BP, 2, 2, nj], F32, tag="hdM")
            hdm = h_pool.tile([BP, 2, 2, nj], F32, tag="hdm")
            hdM2 = h_pool.tile([BP, 2, 2, nj], F32, tag="hdM2")
            hdm2 = h_pool.tile([BP, 2, 2, nj], F32, tag="hdm2")
            hcM = h_pool.tile([BP, 2, 2, nj], F32, tag="hcM")
            hcm = h_pool.tile([BP, 2, 2, nj], F32, tag="hcm")

            d0, d1, d2 = dd[:, :, :, s0], dd[:, :, :, s1], dd[:, :, :, s2]
            cc0, cc1, cc2 = C_[:, :, :, s0], C_[:, :, :, s1], C_[:, :, :, s2]

            nc.vector.tensor_tensor(mk1[:], d1, d0, op=ALU.is_gt)
            nc.vector.tensor_tensor(hdM[:], d0, d1, op=ALU.max)
            nc.vector.tensor_tensor(hdm[:], d0, d1, op=ALU.min)
            nc.vector.tensor_tensor(hdM2[:], hdM[:], d2, op=ALU.max)
            nc.vector.tensor_tensor(hdm2[:], hdm[:], d2, op=ALU.min)
            # L2 masks as nonneg deltas on GpSimd: delta > 0 iff cand 2 wins
            # strictly (exact f32; ties give 0 = keep first occurrence)
            nc.gpsimd.tensor_tensor(dlM[:], hdM2[:], hdM[:], op=ALU.subtract)
            nc.gpsimd.tensor_tensor(dlm[:], hdm[:], hdm2[:], op=ALU.subtract)
            nc.scalar.copy(hcM[:], cc0)
            nc.scalar.copy(hcm[:], cc1)
            # next unit's subtract+abs, emitted here so GpSimd runs this
            # unit's delta-masks first and Scalar runs the seeds first
            if u + 1 < len(UNITS):
                staged = (next_load[1], stage_sub(u + 1, *next_load))
                next_load = stage_load(u + 2) if u + 2 < len(UNITS) else None

            def shift(srct, stag):
                # comp stride padded to NJT so each matmul dst starts on a
                # PSUM bank boundary (nj=511 would misalign comp 1)
                dst_t = ps_pool.tile([BP, 2, NJT], F32, tag=stag)
                for ci in range(2):
                    nc.tensor.matmul(
                        dst_t[:, ci, 0:nj], lhsT=sm[:], rhs=srct[:, ci, 0, :],
                        start=True, stop=True)
                return dst_t[:, :, 0:nj]

            hdME1 = shift(hdM2, "psdM")
            hdmE1 = shift(hdm2, "psdm")

            # ---- vertical d-stage first: fills the DVE while the Scalar
            # seeds and GpSimd delta-masks for the H selects land ----
            mv1M = m_pool.tile([nb, 2, nj], U8, tag="mv1M")
            mv1m = m_pool.tile([nb, 2, nj], U8, tag="mv1m")
            mkVM2 = m_pool.tile([nb, 2, nj], U8, tag="mkVM2")
            mkVm2 = m_pool.tile([nb, 2, nj], U8, tag="mkVm2")
            vdM = v_pool.tile([nb, 2, nj], F32, tag="vdM")
            vdm = v_pool.tile([nb, 2, nj], F32, tag="vdm")
            vcM = v_pool.tile([nb, 2, nj], F32, tag="vcM")
            vcm = v_pool.tile([nb, 2, nj], F32, tag="vcm")

            EM, OM = hdM2[:nb, :, 0, :], hdM2[:nb, :, 1, :]
            Em, Om = hdm2[:nb, :, 0, :], hdm2[:nb, :, 1, :]
            nc.vector.tensor_tensor(mv1M[:], OM, EM, op=ALU.is_gt)
            nc.vector.tensor_tensor(mv1m[:], Om, Em, op=ALU.is_lt)
            nc.vector.tensor_tensor(vdM[:], EM, OM, op=ALU.max)
            nc.vector.tensor_tensor(vdm[:], Em, Om, op=ALU.min)
            nc.vector.tensor_tensor(mkVM2[:], hdME1[:nb], vdM[:], op=ALU.is_gt)
            nc.vector.tensor_tensor(mkVm2[:], hdmE1[:nb], vdm[:], op=ALU.is_lt)

            # ---- horizontal selects ----
            nc.vector.copy_predicated(hcM[:], mk1[:], cc1)
            nc.vector.copy_predicated(hcm[:], mk1[:], cc0)
            nc.vector.copy_predicated(hcM[:], dlM[:].bitcast(U32), cc2)
            nc.vector.copy_predicated(hcm[:], dlm[:].bitcast(U32), cc2)

            hcME1 = shift(hcM, "pscM")
            hcmE1 = shift(hcm, "pscm")

            # ---- vertical selects: candidates u=0 (E0), u=1 (O), u=2 (E1) --
            nc.scalar.copy(vcM[:], hcM[:nb, :, 0, :])
            nc.scalar.copy(vcm[:], hcm[:nb, :, 0, :])
            nc.vector.copy_predicated(vcM[:], mv1M[:], hcM[:nb, :, 1, :])
            nc.vector.copy_predicated(vcm[:], mv1m[:], hcm[:nb, :, 1, :])
            nc.vector.copy_predicated(vcM[:], mkVM2[:], hcME1[:nb])
            nc.vector.copy_predicated(vcm[:], mkVm2[:], hcmE1[:nb])

            # ---- sum on GpSimd; the fp16 duplicate + store is deferred one
            # unit so the scalar queue never delays the next unit's |x| ----
            ss = o_pool.tile([nb, 2, nj], F32, tag="ss")
            nc.gpsimd.tensor_tensor(ss[:], vcM[:], vcm[:], op=ALU.add)
            pending.append((ss, nb, i0, ct))

    flush_out()


@with_exitstack
def _tile_kernel(ctx: ExitStack, tc, outs, ins):
    a, p, n, smat = ins
    outp, outn = outs
    _emit(ctx, tc, a, p, n, smat, outp, outn)


_CACHE = {}


def _build():
    if "nc" in _CACHE:
        return _CACHE["nc"]
    nc = bacc.Bacc(
        "TRN2",
        target_bir_lowering=False,
        debug=False,
        enable_asserts=False,
        num_devices=NCORES,
    )
    a = nc.dram_tensor("a", [SLAB, W], F32, kind="ExternalInput").ap()
    p = nc.dram_tensor("p", [SLAB, W], F32, kind="ExternalInput").ap()
    n = nc.dram_tensor("n", [SLAB, W], F32, kind="ExternalInput").ap()
    smat = nc.dram_tensor("s", [BP, BP], F32, kind="ExternalInput").ap()
    outp = nc.dram_tensor("outp", [OUT_ELEMS], F16, kind="ExternalOutput").ap()
    outn = nc.dram_tensor("outn", [OUT_ELEMS], F16, kind="ExternalOutput").ap()
    with tile.TileContext(nc) as tc:
        _tile_kernel(tc, [outp, outn], [a, p, n, smat])
    nc.compile()
    _CACHE["nc"] = nc
    return nc


def _make_in_maps(anchor, positive, negative):
    smat = np.eye(BP, k=-1, dtype=np.float32)
    in_maps = []
    for k in range(NCORES):
        r0 = OUTR * k
        m = {"s": smat}
        for name, t in (("a", anchor), ("p", positive), ("n", negative)):
            m[name] = np.ascontiguousarray(
                np.asarray(t[r0:r0 + SLAB], dtype=np.float32))
        in_maps.append(m)
    return in_maps


def _host_vrow(anchor, comp, r0):
    """Window-row at image rows r0..r0+2, all 2047 col windows; returns the
    min-sel + max-sel comp values [NJ_TOT] with exact reference semantics."""
    a3 = np.asarray(anchor[r0:r0 + 3], dtype=np.float32)
    c3 = np.asarray(comp[r0:r0 + 3], dtype=np.float32)
    d3 = np.abs(a3 - c3)
    dw = np.lib.stride_tricks.sliding_window_view(d3, 3, axis=1)[:, ::2]
    cw_ = np.lib.stride_tricks.sliding_window_view(c3, 3, axis=1)[:, ::2]
    d9 = dw.transpose(1, 0, 2).reshape(NJ_TOT, 9)
    c9 = cw_.transpose(1, 0, 2).reshape(NJ_TOT, 9)
    ar = np.arange(NJ_TOT)
    return c9[ar, np.argmin(d9, axis=1)] + c9[ar, np.argmax(d9, axis=1)]


def _assemble(results, anchor, positive, negative):
    full = {}
    for name, comp in (("outp", positive), ("outn", negative)):
        out = np.zeros((H, W), np.float32)
        for k in range(NCORES):
            flat = results[k][name]
            cols = []
            for ct, (j0, nj) in enumerate(JTILES):
                wct = 2 * nj
                cols.append(flat[JOFFS[ct]:JOFFS[ct] + OUTR * wct]
                            .astype(np.float32).reshape(OUTR, wct))
            out[OUTR * k:OUTR * (k + 1), 0:2 * NJ_TOT] = np.concatenate(
                cols, axis=1)
        # host-computed window-rows: the last 2 per core (device does 254)
        for k in range(NCORES):
            for iv in (2 * VBLK, 2 * VBLK + 1):   # 254, 255
                gi = VR * k + iv
                if 2 * gi + 3 > H:
                    continue   # core 7 last row pair: overwritten below
                vals = np.repeat(_host_vrow(anchor, comp, 2 * gi), 2)
                out[2 * gi, 0:2 * NJ_TOT] = vals
                out[2 * gi + 1, 0:2 * NJ_TOT] = vals
        comp = np.asarray(comp, dtype=np.float32)
        # cols/rows H-2 replicate the last window's value a third time
        out[:, W - 2] = out[:, W - 3]
        out[H - 2, :] = out[H - 3, :]
        # uncovered last row/col keep clone semantics: min-sel + max-sel = 2c
        out[H - 1, :] = 2.0 * comp[H - 1, :]
        out[:, W - 1] = 2.0 * comp[:, W - 1]
        full[name] = out
    return full["outp"], full["outn"]


def run_on_hw(anchor, positive, negative, trace=False):
    nc = _build()
    in_maps = _make_in_maps(anchor, positive, negative)
    res = bass_utils.run_bass_kernel_spmd(
        nc, in_maps, core_ids=list(range(NCORES)), trace=trace)
    pos, neg = _assemble(res.results, anchor, positive, negative)
    return (pos, neg), res


def kernel(anchor, positive, negative):
    (pos, neg), _ = run_on_hw(anchor, positive, negative, trace=False)
    return pos, neg


# revision 22
# speedup vs baseline: 1.0660x; 1.0660x over previous
"""Trainium2 Bass kernel for nn_DCModule_25451976196444.

Sliding-window (3x3, stride 2) min/max-|anchor-comp| selection pooling:
for each window, pick the comp value where |anchor-comp| is minimal and
where it is maximal; output = sum of the two, broadcast over the window
footprint (last covering window wins).

v3: both comparisons (positive & negative) are batched into every
element-wise op (halves instruction count vs per-comp loops), the L1
horizontal mask is shared between the min and max chains (strict-tie
deviation only, measure-zero on continuous data), |x| and select-seeds
run on the Scalar engine, subtracts on GpSimd, and outputs are stored as
fp16 (halves write traffic; ~2e-4 rel-L2 from quantization, gate 2e-2).
Work is organized in 4 load-units (2 row-blocks x 2 width-halves, ~2MB
DMAs); each unit's load+subtract+abs is emitted one unit AHEAD so the
scalar engine's program order never stalls the vector engine between
iterations.  Stores go through SWDGE (gpsimd.dma_start) which spreads
descriptors across all 16 SDMA engines.

Per core (rows sharded across 8 cores):
  - pair tiles [128, 2, cols]: partition p holds image rows (2p, 2p+1)
  - horizontal pass merges the 3 column candidates per window in f32
    (compares must be f32-exact: fp16 compares flip ~0.5% of windows,
    which fails the error gate)
  - the third vertical candidate (row 2i+2) is the even-plane H-result
    shifted by one partition via TensorE subdiagonal-identity matmul
    (PSUM comp-stride padded to 512 so both matmuls are bank-aligned)
  - vertical pass merges the 3 row candidates; min+max selections are
    summed and column+row duplicated into fp16 tiles for the store
Each core computes 254 of its 256 window-rows; the host computes the last
2 window-rows per core plus the uncovered boundary rows/cols in numpy with
identical f32 semantics.
"""

import numpy as np
from contextlib import ExitStack

import concourse.bass as bass
import concourse.mybir as mybir
import concourse.tile as tile
from concourse import bacc
from concourse import bass_utils
from concourse._compat import with_exitstack

F32 = mybir.dt.float32
F16 = mybir.dt.float16
U8 = mybir.dt.uint8
U32 = mybir.dt.uint32
ALU = mybir.AluOpType
ACTF = mybir.ActivationFunctionType

H = 4096
W = 4096
WS = 3
ST = 2
NCORES = 8
BP = 128                    # partitions per row-block (pair tiles)
NJT = 512                   # window-cols per column tile


def _geom():
    """(Re)compute derived geometry from H/W/BP/NJT (tests patch these)."""
    global OUTR, SLAB, VR, NJ_TOT, VBLK, JTILES, JOFFS, OUT_ELEMS, BLOCKS
    global UNITS, UW
    OUTR = H // NCORES
    SLAB = OUTR
    VR = OUTR // 2
    NJ_TOT = (W - WS) // ST + 1
    VBLK = BP - 1
    assert VR == 2 * VBLK + 2, (VR, VBLK)
    JTILES = []
    j0 = 0
    while j0 < NJ_TOT:
        JTILES.append((j0, min(NJT, NJ_TOT - j0)))
        j0 += NJT
    JOFFS = []
    off = 0
    for (_j, _nj) in JTILES:
        JOFFS.append(off)
        off += OUTR * 2 * _nj
    OUT_ELEMS = off
    BLOCKS = [(0, VBLK), (VBLK, VBLK)]   # device window-rows 0..2*VBLK-1
    # load-units: one (block, jtile) per unit so the working set fits SBUF
    UNITS = []
    for (i0, nb) in BLOCKS:
        for ct in range(len(JTILES)):
            UNITS.append((i0, nb, [ct]))
    UW = max(2 * nj + 1 for (_j, nj) in JTILES)


_geom()


def _emit(ctx: ExitStack, tc, a, p, n, smat, outp, outn):
    nc = tc.nc

    ina_pool = ctx.enter_context(tc.tile_pool(name="ina", bufs=1))
    in_pool = ctx.enter_context(tc.tile_pool(name="in", bufs=2))
    x_pool = ctx.enter_context(tc.tile_pool(name="x", bufs=1))
    d_pool = ctx.enter_context(tc.tile_pool(name="d", bufs=2))
    h_pool = ctx.enter_context(tc.tile_pool(name="h", bufs=1))
    m_pool = ctx.enter_context(tc.tile_pool(name="m", bufs=1))
    v_pool = ctx.enter_context(tc.tile_pool(name="v", bufs=1))
    o_pool = ctx.enter_context(tc.tile_pool(name="o", bufs=2))
    c_pool = ctx.enter_context(tc.tile_pool(name="c", bufs=1))
    ps_pool = ctx.enter_context(tc.tile_pool(name="ps", bufs=1, space="PSUM"))

    sm = c_pool.tile([BP, BP], F32, tag="sm")
    nc.sync.dma_start(sm[:], smat[:])

    def unit_cols(u):
        i0, nb, jts = UNITS[u]
        c0 = 2 * JTILES[jts[0]][0]
        cend = 2 * JTILES[jts[-1]][0] + 2 * JTILES[jts[-1]][1] + 1
        return c0, cend - c0

    def stage_load(u):
        """DMA loads for unit u (emitted one unit ahead)."""
        i0, nb, jts = UNITS[u]
        c0, ucw = unit_cols(u)
        rr = slice(2 * i0, 2 * i0 + 2 * BP)
        cs = slice(c0, c0 + ucw)
        A_ = ina_pool.tile([BP, 2, UW], F32, tag="A")
        C_ = in_pool.tile([BP, 2, 2, UW], F32, tag="C")
        nc.sync.dma_start(
            A_[:, :, 0:ucw], a[rr, cs].rearrange("(q t) w -> q t w", t=2))
        nc.sync.dma_start(
            C_[:, 0, :, 0:ucw], p[rr, cs].rearrange("(q t) w -> q t w", t=2))
        nc.sync.dma_start(
            C_[:, 1, :, 0:ucw], n[rr, cs].rearrange("(q t) w -> q t w", t=2))
        return A_, C_

    def stage_sub(u, A_, C_):
        """x = a - c (GpSimd) and d = |x| (Scalar) for unit u; emitted after
        the current unit's delta-masks so GpSimd produces those first."""
        _i0, _nb, _jts = UNITS[u]
        _c0, ucw = unit_cols(u)
        xx = x_pool.tile([BP, 2, 2, UW], F32, tag="xx")
        nc.gpsimd.tensor_tensor(
            xx[:, 0, :, 0:ucw], A_[:, :, 0:ucw], C_[:, 0, :, 0:ucw],
            op=ALU.subtract)
        nc.gpsimd.tensor_tensor(
            xx[:, 1, :, 0:ucw], A_[:, :, 0:ucw], C_[:, 1, :, 0:ucw],
            op=ALU.subtract)
        dd = d_pool.tile([BP, 2, 2, UW], F32, tag="dd")
        nc.scalar.activation(dd[:, :, :, 0:ucw], xx[:, :, :, 0:ucw], ACTF.Abs)
        return dd

    _l0 = stage_load(0)
    staged = (_l0[1], stage_sub(0, *_l0))
    next_load = stage_load(1) if len(UNITS) > 1 else None
    pending = []   # deferred output stage: (ss, nb, i0, ct) emitted next unit

    def flush_out():
        """Column+row duplicate into fp16 and store via SWDGE."""
        while pending:
            ss, nb, i0, ct = pending.pop(0)
            (j0, nj) = JTILES[ct]
            w = 2 * nj
            vv = o_pool.tile([nb, 2, 2, w], F16, tag="vv")
            nc.scalar.copy(vv[:, :, 0, 0:w - 1:2], ss[:])
            nc.scalar.copy(vv[:, :, 0, 1:w:2], ss[:])
            nc.scalar.copy(vv[:, :, 1, :], vv[:, :, 0, :])
            base = JOFFS[ct] + 2 * i0 * w
            for ci, OUT in ((0, outp), (1, outn)):
                dst = OUT[base:base + 2 * nb * w].rearrange(
                    "(r w) -> r w", w=w)
                nc.gpsimd.dma_start(dst, vv[:, ci])

    for u, (i0, nb, jts) in enumerate(UNITS):
        C_, dd = staged
        flush_out()
        uc0, _ucw = unit_cols(u)

        for ct in jts:
            (j0, nj) = JTILES[ct]
            lc0 = 2 * j0 - uc0          # col offset within the unit tiles
            w = 2 * nj
            s0 = slice(lc0 + 0, lc0 + w - 1, 2)
            s1 = slice(lc0 + 1, lc0 + w, 2)
            s2 = slice(lc0 + 2, lc0 + w + 1, 2)

            # ---- horizontal pass: merge 3 col candidates, both chains ----
            # layout [BP, comp, plane, nj]; candidate order v=0,1,2 with the
            # L1 mask shared between chains (ties deviate, harmless).
            mk1 = m_pool.tile([BP, 2, 2, nj], U8, tag="mk1")
            dlM = m_pool.tile([BP, 2, 2, nj], F32, tag="dlM")
            dlm = m_pool.tile([BP, 2, 2, nj], F32, tag="dlm")
            hdM = h_pool.tile([BP, 2, 2, nj], F32, tag="hdM")
            hdm = h_pool.tile([BP, 2, 2, nj], F32, tag="hdm")
            hdM2 = h_pool.tile([BP, 2, 2, nj], F32, tag="hdM2")
            hdm2 = h_pool.tile([BP, 2, 2, nj], F32, tag="hdm2")
            hcM = h_pool.tile([BP, 2, 2, nj], F32, tag="hcM")
            hcm = h_pool.tile([BP, 2, 2, nj], F32, tag="hcm")

            d0, d1, d2 = dd[:, :, :, s0], dd[:, :, :, s1], dd[:, :, :, s2]
            cc0, cc1, cc2 = C_[:, :, :, s0], C_[:, :, :, s1], C_[:, :, :, s2]

            nc.vector.tensor_tensor(mk1[:], d1, d0, op=ALU.is_gt)
            nc.vector.tensor_tensor(hdM[:], d0, d1, op=ALU.max)
            nc.vector.tensor_tensor(hdm[:], d0, d1, op=ALU.min)
            nc.vector.tensor_tensor(hdM2[:], hdM[:], d2, op=ALU.max)
            nc.vector.tensor_tensor(hdm2[:], hdm[:], d2, op=ALU.min)
            # L2 masks as nonneg deltas on GpSimd: delta > 0 iff cand 2 wins
            # strictly (exact f32; ties give 0 = keep first occurrence)
            nc.gpsimd.tensor_tensor(dlM[:], hdM2[:], hdM[:], op=ALU.subtract)
            nc.gpsimd.tensor_tensor(dlm[:], hdm[:], hdm2[:], op=ALU.subtract)
            nc.scalar.copy(hcM[:], cc0)
            nc.scalar.copy(hcm[:], cc1)
            # next unit's subtract+abs, emitted here so GpSimd runs this
            # unit's delta-masks first and Scalar runs the seeds first
            if u + 1 < len(UNITS):
                staged = (next_load[1], stage_sub(u + 1, *next_load))
                next_load = stage_load(u + 2) if u + 2 < len(UNITS) else None

            def shift(srct, stag):
                # comp stride padded to NJT so each matmul dst starts on a
                # PSUM bank boundary (nj=511 would misalign comp 1)
                dst_t = ps_pool.tile([BP, 2, NJT], F32, tag=stag)
                for ci in range(2):
                    nc.tensor.matmul(
                        dst_t[:, ci, 0:nj], lhsT=sm[:], rhs=srct[:, ci, 0, :],
                        start=True, stop=True)
                return dst_t[:, :, 0:nj]

            hdME1 = shift(hdM2, "psdM")
            hdmE1 = shift(hdm2, "psdm")

            # ---- vertical d-stage first: fills the DVE while the Scalar
            # seeds and GpSimd delta-masks for the H selects land ----
            mv1M = m_pool.tile([nb, 2, nj], U8, tag="mv1M")
            mv1m = m_pool.tile([nb, 2, nj], U8, tag="mv1m")
            mkVM2 = m_pool.tile([nb, 2, nj], U8, tag="mkVM2")
            mkVm2 = m_pool.tile([nb, 2, nj], U8, tag="mkVm2")
            vdM = v_pool.tile([nb, 2, nj], F32, tag="vdM")
            vdm = v_pool.tile([nb, 2, nj], F32, tag="vdm")
            vcM = v_pool.tile([nb, 2, nj], F32, tag="vcM")
            vcm = v_pool.tile([nb, 2, nj], F32, tag="vcm")

            EM, OM = hdM2[:nb, :, 0, :], hdM2[:nb, :, 1, :]
            Em, Om = hdm2[:nb, :, 0, :], hdm2[:nb, :, 1, :]
            nc.vector.tensor_tensor(mv1M[:], OM, EM, op=ALU.is_gt)
            nc.vector.tensor_tensor(mv1m[:], Om, Em, op=ALU.is_lt)
            nc.vector.tensor_tensor(vdM[:], EM, OM, op=ALU.max)
            nc.vector.tensor_tensor(vdm[:], Em, Om, op=ALU.min)
            nc.vector.tensor_tensor(mkVM2[:], hdME1[:nb], vdM[:], op=ALU.is_gt)
            nc.vector.tensor_tensor(mkVm2[:], hdmE1[:nb], vdm[:], op=ALU.is_lt)

            # ---- horizontal selects ----
            nc.vector.copy_predicated(hcM[:], mk1[:], cc1)
            nc.vector.copy_predicated(hcm[:], mk1[:], cc0)
            nc.vector.copy_predicated(hcM[:], dlM[:].bitcast(U32), cc2)
            nc.vector.copy_predicated(hcm[:], dlm[:].bitcast(U32), cc2)

            hcME1 = shift(hcM, "pscM")
            hcmE1 = shift(hcm, "pscm")

            # ---- vertical selects: candidates u=0 (E0), u=1 (O), u=2 (E1) --
            nc.scalar.copy(vcM[:], hcM[:nb, :, 0, :])
            nc.scalar.copy(vcm[:], hcm[:nb, :, 0, :])
            nc.vector.copy_predicated(vcM[:], mv1M[:], hcM[:nb, :, 1, :])
            nc.vector.copy_predicated(vcm[:], mv1m[:], hcm[:nb, :, 1, :])
            nc.vector.copy_predicated(vcM[:], mkVM2[:], hcME1[:nb])
            nc.vector.copy_predicated(vcm[:], mkVm2[:], hcmE1[:nb])

            # ---- sum on GpSimd; the fp16 duplicate + store is deferred one
            # unit so the scalar queue never delays the next unit's |x| ----
            ss = o_pool.tile([nb, 2, nj], F32, tag="ss")
            nc.vector.tensor_tensor(ss[:], vcM[:], vcm[:], op=ALU.add)
            pending.append((ss, nb, i0, ct))

    flush_out()


@with_exitstack
def _tile_kernel(ctx: ExitStack, tc, outs, ins):
    a, p, n, smat = ins
    outp, outn = outs
    _emit(ctx, tc, a, p, n, smat, outp, outn)


_CACHE = {}


def _build():
    if "nc" in _CACHE:
        return _CACHE["nc"]
    nc = bacc.Bacc(
        "TRN2",
        target_bir_lowering=False,
        debug=False,
        enable_asserts=False,
        num_devices=NCORES,
    )
    a = nc.dram_tensor("a", [SLAB, W], F32, kind="ExternalInput").ap()
    p = nc.dram_tensor("p", [SLAB, W], F32, kind="ExternalInput").ap()
    n = nc.dram_tensor("n", [SLAB, W], F32, kind="ExternalInput").ap()
    smat = nc.dram_tensor("s", [BP, BP], F32, kind="ExternalInput").ap()
    outp = nc.dram_tensor("outp", [OUT_ELEMS], F16, kind="ExternalOutput").ap()
    outn = nc.dram_tensor("outn", [OUT_ELEMS], F16, kind="ExternalOutput").ap()
    with tile.TileContext(nc) as tc:
        _tile_kernel(tc, [outp, outn], [a, p, n, smat])
    nc.compile()
    _CACHE["nc"] = nc
    return nc


def _make_in_maps(anchor, positive, negative):
    smat = np.eye(BP, k=-1, dtype=np.float32)
    in_maps = []
    for k in range(NCORES):
        r0 = OUTR * k
        m = {"s": smat}
        for name, t in (("a", anchor), ("p", positive), ("n", negative)):
            m[name] = np.ascontiguousarray(
                np.asarray(t[r0:r0 + SLAB], dtype=np.float32))
        in_maps.append(m)
    return in_maps


def _host_vrow(anchor, comp, r0):
    """Window-row at image rows r0..r0+2, all 2047 col windows; returns the
    min-sel + max-sel comp values [NJ_TOT] with exact reference semantics."""
    a3 = np.asarray(anchor[r0:r0 + 3], dtype=np.float32)
    c3 = np.asarray(comp[r0:r0 + 3], dtype=np.float32)
    d3 = np.abs(a3 - c3)
    dw = np.lib.stride_tricks.sliding_window_view(d3, 3, axis=1)[:, ::2]
    cw_ = np.lib.stride_tricks.sliding_window_view(c3, 3, axis=1)[:, ::2]
    d9 = dw.transpose(1, 0, 2).reshape(NJ_TOT, 9)
    c9 = cw_.transpose(1, 0, 2).reshape(NJ_TOT, 9)
    ar = np.arange(NJ_TOT)
    return c9[ar, np.argmin(d9, axis=1)] + c9[ar, np.argmax(d9, axis=1)]


def _assemble(results, anchor, positive, negative):
    full = {}
    for name, comp in (("outp", positive), ("outn", negative)):
        out = np.zeros((H, W), np.float32)
        for k in range(NCORES):
            flat = results[k][name]
            cols = []
            for ct, (j0, nj) in enumerate(JTILES):
                wct = 2 * nj
                cols.append(flat[JOFFS[ct]:JOFFS[ct] + OUTR * wct]
                            .astype(np.float32).reshape(OUTR, wct))
            out[OUTR * k:OUTR * (k + 1), 0:2 * NJ_TOT] = np.concatenate(
                cols, axis=1)
        # host-computed window-rows: the last 2 per core (device does 254)
        for k in range(NCORES):
            for iv in (2 * VBLK, 2 * VBLK + 1):   # 254, 255
                gi = VR * k + iv
                if 2 * gi + 3 > H:
                    continue   # core 7 last row pair: overwritten below
                vals = np.repeat(_host_vrow(anchor, comp, 2 * gi), 2)
                out[2 * gi, 0:2 * NJ_TOT] = vals
                out[2 * gi + 1, 0:2 * NJ_TOT] = vals
        comp = np.asarray(comp, dtype=np.float32)
        # cols/rows H-2 replicate the last window's value a third time
        out[:, W - 2] = out[:, W - 3]
        out[H - 2, :] = out[H - 3, :]
        # uncovered last row/col keep clone semantics: min-sel + max-sel = 2c
        out[H - 1, :] = 2.0 * comp[H - 1, :]
        out[:, W - 1] = 2.0 * comp[:, W - 1]
        full[name] = out
    return full["outp"], full["outn"]


def run_on_hw(anchor, positive, negative, trace=False):
    nc = _build()
    in_maps = _make_in_maps(anchor, positive, negative)
    res = bass_utils.run_bass_kernel_spmd(
        nc, in_maps, core_ids=list(range(NCORES)), trace=trace)
    pos, neg = _assemble(res.results, anchor, positive, negative)
    return (pos, neg), res


def kernel(anchor, positive, negative):
    (pos, neg), _ = run_on_hw(anchor, positive, negative, trace=False)
    return pos, neg
